# revision 1
# baseline (speedup 1.0000x reference)
"""Trainium2 Bass kernel: per-vertex neighbor mean+max gather-reduce.

reference: out[v] = concat(sum_k x[idxs[v,k]] / K, max_k x[idxs[v,k]])
  x: [100000, 64] f32, idxs: [100000, 32] int64 -> out [100000, 128] f32

Strategy (8 NeuronCores):
  - Shard vertices (rows of idxs) across the 8 cores; replicate x to every
    core's HBM (25.6 MB).
  - Per core: for each 128-vertex tile, load the [128, 32] int32 index tile,
    indirect-DMA-gather the 32 neighbor rows (256 B each) of every vertex
    into a [128, 32*64] SBUF tile (vertex = partition), then reduce over the
    32-neighbor axis on the vector engine (sum + max), scale sum by 1/K on
    the scalar engine, and DMA the [128, 128] result out.
"""

import numpy as np

import concourse.bacc as bacc
import concourse.bass as bass
import concourse.mybir as mybir
import concourse.tile as tile
from concourse.bass_utils import run_bass_kernel_spmd

V, K, F = 100000, 32, 64
NCORES = 8
P = 128
VS_RAW = V // NCORES            # 12500 vertices per core
TILES = -(-VS_RAW // P)         # 98
VS = TILES * P                  # 12544 (padded)

TRACE = False                   # test.py flips this to capture an NTFF profile
_cache = {}


def _build(v=V, vs=VS, f=F, k=K, bufs=8):
    nc = bacc.Bacc("TRN2", dynamic_dma_scratch_size=131072)
    x_d = nc.dram_tensor("x", [v, f], mybir.dt.float32, kind="ExternalInput")
    idx_d = nc.dram_tensor("idxs", [vs, k], mybir.dt.int32, kind="ExternalInput")
    out_d = nc.dram_tensor("out", [vs, 2 * f], mybir.dt.float32, kind="ExternalOutput")
    ntiles = vs // P
    with tile.TileContext(nc) as tc:
        with tc.tile_pool(name="pool", bufs=bufs) as pool:
            for t in range(ntiles):
                idx_tile = pool.tile([P, k], mybir.dt.int32)
                nc.sync.dma_start(out=idx_tile[:], in_=idx_d[t * P:(t + 1) * P, :])
                g = pool.tile([P, k * f], mybir.dt.float32)
                # HW contract: one index per partition per call, each index
                # streaming the dest partition's free extent. One call per k.
                for kk in range(k):
                    nc.gpsimd.indirect_dma_start(
                        out=g[:, kk * f:(kk + 1) * f],
                        out_offset=None,
                        in_=x_d[:],
                        in_offset=bass.IndirectOffsetOnAxis(
                            ap=idx_tile[:, kk:kk + 1], axis=0),
                    )
                o = pool.tile([P, 2 * f], mybir.dt.float32)
                gv = g[:].rearrange("p (k f) -> p f k", k=k, f=f)
                nc.vector.tensor_reduce(
                    out=o[:, 0:f], in_=gv,
                    axis=mybir.AxisListType.X, op=mybir.AluOpType.add,
                )
                nc.vector.tensor_reduce(
                    out=o[:, f:2 * f], in_=gv,
                    axis=mybir.AxisListType.X, op=mybir.AluOpType.max,
                )
                nc.scalar.mul(o[:, 0:f], o[:, 0:f], 1.0 / k)
                nc.sync.dma_start(out=out_d[t * P:(t + 1) * P, :], in_=o[:])
    nc.compile()
    return nc


def kernel(x, idxs):
    x = np.ascontiguousarray(np.asarray(x), dtype=np.float32)
    idxs = np.asarray(idxs)
    assert x.shape == (V, F) and idxs.shape == (V, K)

    idx32 = np.zeros((NCORES, VS, K), np.int32)
    idx32[:, :VS_RAW] = idxs.astype(np.int32).reshape(NCORES, VS_RAW, K)

    if "nc" not in _cache:
        _cache["nc"] = _build()
    in_maps = [
        {"x": x, "idxs": np.ascontiguousarray(idx32[c])} for c in range(NCORES)
    ]
    res = run_bass_kernel_spmd(
        _cache["nc"], in_maps, core_ids=list(range(NCORES)), trace=TRACE,
    )
    kernel.last_results = res
    return np.concatenate(
        [r["out"][:VS_RAW] for r in res.results], axis=0
    )

